# revision 9
# baseline (speedup 1.0000x reference)
"""Trainium2 Bass kernel for nn_CoAttentionPara.

Reference computation per batch b (S=512, R=196, E=1024, K=512):
    affi = (T @ Wb) @ I^T                  [S, R]
    tWq  = T @ Wq                          [S, K]
    iWv  = I @ Wv                          [R, K]
    Hv   = tanh(iWv + affi^T @ tWq)        [R, K]
    av   = softmax(Hv @ Whv + bhv)         [R]
    Hq   = tanh(tWq + affi @ iWv)          [S, K]
    aq   = softmax(Hq @ Whq + bhq)         [S]
    out  = (aq[:, None] * T, av[:, None] * I)

Kernel strategy:
  - Data-parallel over batch: 8 cores x 4 batches, weights replicated.
  - affi is computed via the cheaper association T @ (Wb @ I^T)
    (E->R contraction shrinks the intermediate): P = Wb @ I^T [E, R],
    affi^T = P^T @ T^T via PE, affi from affi^T via PE transpose.
  - All E-contractions need T/I with E on partitions, so T^T/I^T are
    built on-chip with PE transposes (DMA transpose is 2-byte only).
  - Hv/Hq are accumulated fully inside PSUM banks in transposed [K, *]
    layout (matmul accumulation + PE-transposed addend blocks), so the
    tanh is a single ACT pass and the Whv/Whq dot contracts on
    partitions via M=1 matmuls.
  - bhv/bhq are scalars broadcast over the softmax axis, so they cancel
    in softmax and are ignored.
"""

import numpy as np

_B, _S, _R, _E, _K = 32, 512, 196, 1024, 512
_NCORES = 8
_BPC = _B // _NCORES  # batches per core
_EB, _SB, _NB, _KB = _E // 128, _S // 128, 2, _K // 128
_NSZ = [128, _R - 128]  # R=196 -> partition tile sizes [128, 68]

_cached_nc = None
_DEBUG = False  # adds dbg_* outputs for batch 0 intermediates


def _emit(tc, ctx, aps):
    import concourse.bass as bass
    import concourse.mybir as mybir
    from concourse.masks import make_identity

    fp32 = mybir.dt.float32
    Tanh = mybir.ActivationFunctionType.Tanh
    Exp = mybir.ActivationFunctionType.Exp

    nc = tc.nc
    tF, iF, Wb, Wq, Wv, Whv, Whq, tOut, iOut = aps
    dbg = {}
    if _DEBUG:
        def _mk(name, shape):
            dbg[name] = nc.dram_tensor("dbg_" + name, shape, fp32,
                                       kind="ExternalOutput").ap()
        _mk("wbT", [128, _EB * _E])
        _mk("tT", [128, _EB * _S])
        _mk("iT", [128, _EB * _R])
        _mk("p", [128, _EB * _R])
        _mk("afT", [128, _NB * _S])
        _mk("af", [128, _SB * _R])
        _mk("twq", [128, _SB * _K])
        _mk("iwv", [128, _NB * _K])
        _mk("hvT", [128, _KB * _R])
        _mk("hqT", [128, _KB * _S])
        _mk("avr", [1, _R])
        _mk("aqr", [1, _S])
        _mk("avcol", [128, _NB])
        _mk("aqcol", [128, _SB])

    const = ctx.enter_context(tc.tile_pool(name="const", bufs=1))
    ldw = ctx.enter_context(tc.tile_pool(name="ldw", bufs=2))
    tp = ctx.enter_context(tc.tile_pool(name="tp", bufs=2))
    ip = ctx.enter_context(tc.tile_pool(name="ip", bufs=2))
    ttp = ctx.enter_context(tc.tile_pool(name="ttp", bufs=1))
    itp = ctx.enter_context(tc.tile_pool(name="itp", bufs=1))
    mid1 = ctx.enter_context(tc.tile_pool(name="mid1", bufs=1))
    mid2 = ctx.enter_context(tc.tile_pool(name="mid2", bufs=2))
    smalls = ctx.enter_context(tc.tile_pool(name="smalls", bufs=1))
    psum_tr = ctx.enter_context(tc.tile_pool(name="psum_tr", bufs=2, space="PSUM"))
    psum_mm = ctx.enter_context(tc.tile_pool(name="psum_mm", bufs=4, space="PSUM"))
    psum_lg = ctx.enter_context(tc.tile_pool(name="psum_lg", bufs=2, space="PSUM"))

    ident = const.tile([128, 128], fp32)
    make_identity(nc, ident)

    # --- shared weights ---
    wq_sb = const.tile([128, _EB * _K], fp32, tag="wq_sb")  # [e%128, (eb,k)]
    wv_sb = const.tile([128, _EB * _K], fp32, tag="wv_sb")
    for eb in range(_EB):
        nc.sync.dma_start(
            out=wq_sb[:, eb * _K:(eb + 1) * _K], in_=Wq[eb * 128:(eb + 1) * 128, :])
        nc.sync.dma_start(
            out=wv_sb[:, eb * _K:(eb + 1) * _K], in_=Wv[eb * 128:(eb + 1) * 128, :])
    whv_sb = const.tile([128, _KB], fp32, tag="whv_sb")
    whq_sb = const.tile([128, _KB], fp32, tag="whq_sb")
    for kb in range(_KB):
        nc.sync.dma_start(out=whv_sb[:, kb:kb + 1], in_=Whv[kb * 128:(kb + 1) * 128, :])
        nc.sync.dma_start(out=whq_sb[:, kb:kb + 1], in_=Whq[kb * 128:(kb + 1) * 128, :])

    # Wb^T [r, (rb? no: (rb-major block) e] : wbT[:, rb*_E + eb*128 : +128] is
    # the [r in rb-block (128 partitions), e in eb-block (128 cols)] tile.
    wbT_sb = const.tile([128, _EB * _E], fp32, tag="wbT_sb")
    for eb in range(_EB):
        wb_nat = ldw.tile([128, _E], fp32)  # [e in eb-block, r]
        nc.sync.dma_start(out=wb_nat, in_=Wb[eb * 128:(eb + 1) * 128, :])
        for rb in range(_EB):
            ps = psum_tr.tile([128, 128], fp32, tag="ps_tr")
            nc.tensor.matmul(ps, lhsT=wb_nat[:, rb * 128:(rb + 1) * 128],
                             rhs=ident, is_transpose=True)
            nc.vector.tensor_copy(wbT_sb[:, rb * _E + eb * 128:rb * _E + (eb + 1) * 128], ps)
    if _DEBUG:
        nc.sync.dma_start(out=dbg["wbT"], in_=wbT_sb)

    def _dump(b, name, ap):
        if _DEBUG and b == 0:
            nc.sync.dma_start(out=dbg[name], in_=ap)

    for b in range(_BPC):
        # --- load T, I (natural layout, kept for the final scaling) ---
        t_nat = tp.tile([128, _SB * _E], fp32, tag="t_nat")  # [s%128, (sb,e)]
        for sb in range(_SB):
            nc.sync.dma_start(out=t_nat[:, sb * _E:(sb + 1) * _E],
                              in_=tF[b, sb * 128:(sb + 1) * 128, :])
        i_nat = ip.tile([128, _NB * _E], fp32, tag="i_nat")  # [n%128, (nb,e)]
        for nb in range(_NB):
            nsz = _NSZ[nb]
            nc.sync.dma_start(out=i_nat[:nsz, nb * _E:(nb + 1) * _E],
                              in_=iF[b, nb * 128:nb * 128 + nsz, :])

        # --- T^T [e%128, (eb,s)], I^T [e%128, (eb,n)] via PE transposes ---
        tT = ttp.tile([128, _EB * _S], fp32, tag="tT")
        for eb in range(_EB):
            for sb in range(_SB):
                ps = psum_tr.tile([128, 128], fp32, tag="ps_tr")
                nc.tensor.matmul(ps, lhsT=t_nat[:, sb * _E + eb * 128:sb * _E + (eb + 1) * 128],
                                 rhs=ident, is_transpose=True)
                nc.vector.tensor_copy(tT[:, eb * _S + sb * 128:eb * _S + (sb + 1) * 128], ps)
        iT = itp.tile([128, _EB * _R], fp32, tag="iT")
        for eb in range(_EB):
            for nb in range(_NB):
                nsz = _NSZ[nb]
                ps = psum_tr.tile([128, 128], fp32, tag="ps_tr")
                nc.tensor.matmul(ps[:, :nsz],
                                 lhsT=i_nat[:nsz, nb * _E + eb * 128:nb * _E + (eb + 1) * 128],
                                 rhs=ident[:nsz, :nsz], is_transpose=True)
                nc.vector.tensor_copy(
                    iT[:, eb * _R + nb * 128:eb * _R + nb * 128 + nsz], ps[:, :nsz])

        _dump(b, "tT", tT)
        _dump(b, "iT", iT)
        # --- P = Wb @ I^T  [e%128, (eb,n)] ---
        p_sb = mid1.tile([128, _EB * _R], fp32, tag="p_sb")
        for eb in range(_EB):
            pp = psum_mm.tile([128, _K], fp32, tag="mm")
            for rb in range(_EB):
                nc.tensor.matmul(pp[:, :_R],
                                 lhsT=wbT_sb[:, rb * _E + eb * 128:rb * _E + (eb + 1) * 128],
                                 rhs=iT[:, rb * _R:(rb + 1) * _R],
                                 start=(rb == 0), stop=(rb == _EB - 1))
            nc.vector.tensor_copy(p_sb[:, eb * _R:(eb + 1) * _R], pp[:, :_R])

        _dump(b, "p", p_sb)
        # --- affi^T = P^T @ T^T  [n%128, (nb,s)] ---
        afT_sb = mid2.tile([128, _NB * _S], fp32, tag="afT_sb")
        for nb in range(_NB):
            nsz = _NSZ[nb]
            pa = psum_mm.tile([128, _K], fp32, tag="mm")
            for eb in range(_EB):
                nc.tensor.matmul(pa[:nsz, :_S],
                                 lhsT=p_sb[:, eb * _R + nb * 128:eb * _R + nb * 128 + nsz],
                                 rhs=tT[:, eb * _S:(eb + 1) * _S],
                                 start=(eb == 0), stop=(eb == _EB - 1))
            nc.vector.tensor_copy(afT_sb[:nsz, nb * _S:(nb + 1) * _S], pa[:nsz, :_S])

        _dump(b, "afT", afT_sb)
        # --- affi [s%128, (sb,n)] via PE transpose of affi^T ---
        af_sb = mid2.tile([128, _SB * _R], fp32, tag="af_sb")
        for sb in range(_SB):
            for nb in range(_NB):
                nsz = _NSZ[nb]
                ps = psum_tr.tile([128, 128], fp32, tag="ps_tr")
                nc.tensor.matmul(ps[:, :nsz],
                                 lhsT=afT_sb[:nsz, nb * _S + sb * 128:nb * _S + (sb + 1) * 128],
                                 rhs=ident[:nsz, :nsz], is_transpose=True)
                nc.vector.tensor_copy(
                    af_sb[:, sb * _R + nb * 128:sb * _R + nb * 128 + nsz], ps[:, :nsz])

        _dump(b, "af", af_sb)
        # --- tWq = T @ Wq  [s%128, (sb,k)] ---
        twq_sb = mid1.tile([128, _SB * _K], fp32, tag="twq_sb")
        for sb in range(_SB):
            pt = psum_mm.tile([128, _K], fp32, tag="mm")
            for eb in range(_EB):
                nc.tensor.matmul(pt,
                                 lhsT=tT[:, eb * _S + sb * 128:eb * _S + (sb + 1) * 128],
                                 rhs=wq_sb[:, eb * _K:(eb + 1) * _K],
                                 start=(eb == 0), stop=(eb == _EB - 1))
            nc.vector.tensor_copy(twq_sb[:, sb * _K:(sb + 1) * _K], pt)

        _dump(b, "twq", twq_sb)
        # --- iWv = I @ Wv  [n%128, (nb,k)] ---
        iwv_sb = mid2.tile([128, _NB * _K], fp32, tag="iwv_sb")
        for nb in range(_NB):
            nsz = _NSZ[nb]
            pi = psum_mm.tile([128, _K], fp32, tag="mm")
            for eb in range(_EB):
                nc.tensor.matmul(pi[:nsz, :],
                                 lhsT=iT[:, eb * _R + nb * 128:eb * _R + nb * 128 + nsz],
                                 rhs=wv_sb[:, eb * _K:(eb + 1) * _K],
                                 start=(eb == 0), stop=(eb == _EB - 1))
            nc.vector.tensor_copy(iwv_sb[:nsz, nb * _K:(nb + 1) * _K], pi[:nsz, :])

        _dump(b, "iwv", iwv_sb)
        # --- Hv^T = tanh(iWv^T + tWq^T @ affi)  [k%128, (kb,n)] ---
        # NOTE: start=True clears has_written for the WHOLE bank, so the
        # accumulation group must open with one matmul covering the full
        # footprint; the PE-transposed addend blocks follow with start=False.
        hvT_sb = mid2.tile([128, _KB * _R], fp32, tag="hvT_sb")
        for kb in range(_KB):
            ph = psum_mm.tile([128, _K], fp32, tag="mm")
            for sb in range(_SB):
                nc.tensor.matmul(ph[:, :_R],
                                 lhsT=twq_sb[:, sb * _K + kb * 128:sb * _K + (kb + 1) * 128],
                                 rhs=af_sb[:, sb * _R:(sb + 1) * _R],
                                 start=(sb == 0), stop=False, skip_group_check=True)
            for nb in range(_NB):
                nsz = _NSZ[nb]
                nc.tensor.matmul(ph[:, nb * 128:nb * 128 + nsz],
                                 lhsT=iwv_sb[:nsz, nb * _K + kb * 128:nb * _K + (kb + 1) * 128],
                                 rhs=ident[:nsz, :nsz], is_transpose=True,
                                 start=False, stop=(nb == _NB - 1), skip_group_check=True)
            nc.scalar.activation(hvT_sb[:, kb * _R:(kb + 1) * _R], ph[:, :_R], Tanh)

        _dump(b, "hvT", hvT_sb)
        # --- Hq^T = tanh(tWq^T + iWv^T @ affi^T)  [k%128, (kb,s)] ---
        hqT_sb = mid1.tile([128, _KB * _S], fp32, tag="hqT_sb")
        for kb in range(_KB):
            ph = psum_mm.tile([128, _K], fp32, tag="mm")
            for nb in range(_NB):
                nsz = _NSZ[nb]
                nc.tensor.matmul(ph[:, :_S],
                                 lhsT=iwv_sb[:nsz, nb * _K + kb * 128:nb * _K + (kb + 1) * 128],
                                 rhs=afT_sb[:nsz, nb * _S:(nb + 1) * _S],
                                 start=(nb == 0), stop=False, skip_group_check=True)
            for sb in range(_SB):
                nc.tensor.matmul(ph[:, sb * 128:(sb + 1) * 128],
                                 lhsT=twq_sb[:, sb * _K + kb * 128:sb * _K + (kb + 1) * 128],
                                 rhs=ident, is_transpose=True,
                                 start=False, stop=(sb == _SB - 1), skip_group_check=True)
            nc.scalar.activation(hqT_sb[:, kb * _S:(kb + 1) * _S], ph[:, :_S], Tanh)

        _dump(b, "hqT", hqT_sb)
        # --- av = softmax_n(Hv @ Whv) ---
        plv = psum_lg.tile([1, _K], fp32, tag="lg")
        for kb in range(_KB):
            nc.tensor.matmul(plv[:, :_R], lhsT=whv_sb[:, kb:kb + 1],
                             rhs=hvT_sb[:, kb * _R:(kb + 1) * _R],
                             start=(kb == 0), stop=(kb == _KB - 1))
        mxv = smalls.tile([1, 1], fp32, tag="mxv")
        nc.vector.reduce_max(out=mxv, in_=plv[:, :_R], axis=mybir.AxisListType.X)
        nmxv = smalls.tile([1, 1], fp32, tag="nmxv")
        nc.vector.tensor_scalar_mul(nmxv, mxv, -1.0)
        ev = smalls.tile([1, _R], fp32, tag="ev")
        sv = smalls.tile([1, 1], fp32, tag="sv")
        nc.scalar.activation(ev, plv[:, :_R], Exp, bias=nmxv, accum_out=sv)
        rv = smalls.tile([1, 1], fp32, tag="rv")
        nc.vector.reciprocal(rv, sv)
        avr = smalls.tile([1, _R], fp32, tag="avr")
        nc.vector.tensor_scalar_mul(avr, ev, rv)
        avcol = smalls.tile([128, _NB], fp32, tag="avcol")
        for nb in range(_NB):
            nsz = _NSZ[nb]
            ps = psum_tr.tile([128, 128], fp32, tag="ps_tr")
            nc.tensor.matmul(ps[:nsz, :1], lhsT=avr[:, nb * 128:nb * 128 + nsz],
                             rhs=ident[:1, :1], is_transpose=True)
            nc.vector.tensor_copy(avcol[:nsz, nb:nb + 1], ps[:nsz, :1])

        _dump(b, "avr", avr)
        _dump(b, "avcol", avcol)
        # --- aq = softmax_s(Hq @ Whq) ---
        plq = psum_lg.tile([1, _K], fp32, tag="lg")
        for kb in range(_KB):
            nc.tensor.matmul(plq[:, :_S], lhsT=whq_sb[:, kb:kb + 1],
                             rhs=hqT_sb[:, kb * _S:(kb + 1) * _S],
                             start=(kb == 0), stop=(kb == _KB - 1))
        mxq = smalls.tile([1, 1], fp32, tag="mxq")
        nc.vector.reduce_max(out=mxq, in_=plq[:, :_S], axis=mybir.AxisListType.X)
        nmxq = smalls.tile([1, 1], fp32, tag="nmxq")
        nc.vector.tensor_scalar_mul(nmxq, mxq, -1.0)
        eq = smalls.tile([1, _S], fp32, tag="eq")
        sq = smalls.tile([1, 1], fp32, tag="sq")
        nc.scalar.activation(eq, plq[:, :_S], Exp, bias=nmxq, accum_out=sq)
        rq = smalls.tile([1, 1], fp32, tag="rq")
        nc.vector.reciprocal(rq, sq)
        aqr = smalls.tile([1, _S], fp32, tag="aqr")
        nc.vector.tensor_scalar_mul(aqr, eq, rq)
        aqcol = smalls.tile([128, _SB], fp32, tag="aqcol")
        for sb in range(_SB):
            ps = psum_tr.tile([128, 128], fp32, tag="ps_tr")
            nc.tensor.matmul(ps[:, :1], lhsT=aqr[:, sb * 128:(sb + 1) * 128],
                             rhs=ident[:1, :1], is_transpose=True)
            nc.vector.tensor_copy(aqcol[:, sb:sb + 1], ps[:, :1])

        _dump(b, "aqr", aqr)
        _dump(b, "aqcol", aqcol)
        # --- scale features and store ---
        for sb in range(_SB):
            nc.vector.tensor_scalar_mul(t_nat[:, sb * _E:(sb + 1) * _E],
                                        t_nat[:, sb * _E:(sb + 1) * _E],
                                        aqcol[:, sb:sb + 1])
            nc.sync.dma_start(out=tOut[b, sb * 128:(sb + 1) * 128, :],
                              in_=t_nat[:, sb * _E:(sb + 1) * _E])
        for nb in range(_NB):
            nsz = _NSZ[nb]
            nc.vector.tensor_scalar_mul(i_nat[:nsz, nb * _E:(nb + 1) * _E],
                                        i_nat[:nsz, nb * _E:(nb + 1) * _E],
                                        avcol[:nsz, nb:nb + 1])
            nc.sync.dma_start(out=iOut[b, nb * 128:nb * 128 + nsz, :],
                              in_=i_nat[:nsz, nb * _E:(nb + 1) * _E])


def _build_program():
    from contextlib import ExitStack

    import concourse.mybir as mybir
    import concourse.tile as tile
    from concourse import bacc

    fp32 = mybir.dt.float32
    nc = bacc.Bacc("TRN2", target_bir_lowering=False, debug=False,
                   num_devices=_NCORES)
    aps = [
        nc.dram_tensor("tF", [_BPC, _S, _E], fp32, kind="ExternalInput").ap(),
        nc.dram_tensor("iF", [_BPC, _R, _E], fp32, kind="ExternalInput").ap(),
        nc.dram_tensor("Wb", [_E, _E], fp32, kind="ExternalInput").ap(),
        nc.dram_tensor("Wq", [_E, _K], fp32, kind="ExternalInput").ap(),
        nc.dram_tensor("Wv", [_E, _K], fp32, kind="ExternalInput").ap(),
        nc.dram_tensor("Whv", [_K, 1], fp32, kind="ExternalInput").ap(),
        nc.dram_tensor("Whq", [_K, 1], fp32, kind="ExternalInput").ap(),
        nc.dram_tensor("tOut", [_BPC, _S, _E], fp32, kind="ExternalOutput").ap(),
        nc.dram_tensor("iOut", [_BPC, _R, _E], fp32, kind="ExternalOutput").ap(),
    ]
    with tile.TileContext(nc) as tc:
        with ExitStack() as ctx:
            _emit(tc, ctx, aps)
    nc.compile()
    return nc


def _get_program():
    global _cached_nc
    if _cached_nc is None:
        _cached_nc = _build_program()
    return _cached_nc


def kernel(tFeature, iFeature, Wb, Wq, Wv, Whv, bhv, Whq, bhq, _trace=False):
    # bhv/bhq are scalar shifts of softmax logits -> mathematically
    # cancelled by the softmax; not used on-device.
    from concourse.bass_utils import run_bass_kernel_spmd

    nc = _get_program()
    tFeature = np.ascontiguousarray(np.asarray(tFeature), dtype=np.float32)
    iFeature = np.ascontiguousarray(np.asarray(iFeature), dtype=np.float32)
    shared = {
        "Wb": np.ascontiguousarray(np.asarray(Wb), dtype=np.float32),
        "Wq": np.ascontiguousarray(np.asarray(Wq), dtype=np.float32),
        "Wv": np.ascontiguousarray(np.asarray(Wv), dtype=np.float32),
        "Whv": np.ascontiguousarray(np.asarray(Whv), dtype=np.float32).reshape(_K, 1),
        "Whq": np.ascontiguousarray(np.asarray(Whq), dtype=np.float32).reshape(_K, 1),
    }
    in_maps = []
    for c in range(_NCORES):
        lo, hi = c * _BPC, (c + 1) * _BPC
        in_maps.append({"tF": tFeature[lo:hi], "iF": iFeature[lo:hi], **shared})
    res = run_bass_kernel_spmd(nc, in_maps, core_ids=list(range(_NCORES)),
                               trace=_trace)
    tOut = np.concatenate([res.results[c]["tOut"] for c in range(_NCORES)], axis=0)
    iOut = np.concatenate([res.results[c]["iOut"] for c in range(_NCORES)], axis=0)
    if _trace:
        return (tOut, iOut), res
    return (tOut, iOut)
